# revision 31
# baseline (speedup 1.0000x reference)
"""Bahdanau attention on 8 Trainium2 NeuronCores (Bass/Tile).

Problem:  S=2048, B=32, D=1024, E2=1024
  ws  = dec @ Wb.T                       [B, D]
  WH  = enc @ Wc.T                       [S, B, D]
  sc  = tanh(WH + ws) . Wa               [S, B]
  at  = softmax(sc, axis=0)              [S, B]
  out = einsum('sb,sbe->be', at, enc)[None]   [1, B, 2E]

Sharding: data-parallel over batch B across 8 cores (4 batches/core);
weights replicated. Softmax axis (S) stays core-local.

Dataflow (v2, transposed WH):
  WH is computed TRANSPOSED per s-tile: psum [128s, 512d] with the enc^T
  chunk as the stationary operand and Wc^T as the moving operand. With s on
  partitions and d on the free axis, the whole score reduction moves off
  the PE: ws is added as a free-axis row broadcast (DVE), tanh on ACT, and
  score[s] = sum_d Wa_d tanh(...) is a fused multiply+reduce on DVE
  (tensor_tensor_reduce with chained accumulate across the two d-halves).
  exp then lands directly in column form [128s, 1] (ACT, with accum_out
  providing softmax-Z partials), which feeds the context matmuls as lhsT
  with no transpose/columnize step. The context contraction accumulates
  unnormalized in psum over the four 128-s blocks, folded into an SBUF
  accumulator by DVE, scaled once by 1/Z per batch.

Precision: everything bf16 except psum/accumulators (f32). The first NF8
e-dims of the WH contraction run as fp8e4 (e4m3) pairs with
MatmulPerfMode.DoubleRow, which processes K=256 per matmul at the same
per-matmul cost as K=128 bf16 (HW-measured 259 ns/MM for both at N=512,
LDWEIGHTS fully hidden). exp needs no max-subtraction: |score| <= ~26 so
exp fits fp32/bf16 range comfortably and softmax is shift-invariant.

The PE is the bottleneck (>90% busy); instruction emission order doubles
as the schedule: each tile's context matmuls are emitted after the NEXT
tile's WH matmuls so the PE never waits on the ACT exp; ~20 warmup
matmuls on garbage data ramp the PE p-state during the startup DMAs.
"""

import numpy as np
import ml_dtypes

S, B, D, E2 = 2048, 32, 1024, 1024
NCORES = 8
BL = B // NCORES          # batches per core
ST = 512                  # s-tile size
NST = S // ST             # s-tiles per batch
NSB = ST // 128           # 128-row s-blocks per s-tile
DC = D // 128             # d chunks (ws path)

import os
NF8 = int(os.environ.get("K_NF8", "512"))  # leading e-dims in fp8 DoubleRow
NP8 = NF8 // 256          # fp8 k-pairs
EC16 = (E2 - NF8) // 128  # bf16 e-chunks
_NO_TTR = bool(int(os.environ.get("K_NO_TTR", "1")))  # ttr crashes HW runtime
_NO_WARM = bool(int(os.environ.get("K_NO_WARM", "0")))
_NO_RT = bool(int(os.environ.get("K_NO_RT", "0")))    # skip DRAM ws round-trip
_LIMIT = int(os.environ.get("K_LIMIT", "99"))         # max tiles emitted
_AMR = bool(int(os.environ.get("K_AMR", "1")))        # fused affine_mul_reduce
_GPADD = bool(int(os.environ.get("K_GPADD", "0")))    # ws-add on gpsimd
_WSMM = bool(int(os.environ.get("K_WSMM", "1")))      # fold ws into psum via PE

_CACHE = {}


def _build_nc():
    import concourse.bacc as bacc
    import concourse.tile as tile
    from concourse import mybir

    f32 = mybir.dt.float32
    bf16 = mybir.dt.bfloat16
    f8 = mybir.dt.float8e4
    TANH = mybir.ActivationFunctionType.Tanh
    EXP = mybir.ActivationFunctionType.Exp
    X = mybir.AxisListType.X
    MUL = mybir.AluOpType.mult
    ADD = mybir.AluOpType.add
    DR = mybir.MatmulPerfMode.DoubleRow

    nc = bacc.Bacc()
    # host-prepped layouts (see _prep_inputs)
    if NP8:
        enct8_h = nc.declare_dram_parameter("enct8", [128, NP8, 2, BL, S], f8, isOutput=False)
        wct8_h = nc.declare_dram_parameter("wct8", [128, NP8, 2, D], f8, isOutput=False)
    if EC16:
        enct16_h = nc.declare_dram_parameter("enct16", [128, EC16, BL, S], bf16, isOutput=False)
        wct16_h = nc.declare_dram_parameter("wct16", [128, EC16, D], bf16, isOutput=False)
    encn_h = nc.declare_dram_parameter("encn", [128, BL, S // 128, E2], bf16, isOutput=False)
    dect_h = nc.declare_dram_parameter("dect", [128, DC, BL], bf16, isOutput=False)
    wbt_h = nc.declare_dram_parameter("wbt", [128, DC, D], bf16, isOutput=False)
    wab_h = nc.declare_dram_parameter("wab", [128, D], bf16, isOutput=False)
    outp = nc.declare_dram_parameter("out", [BL, E2], f32, isOutput=True)
    wsx_h = nc.declare_dram_parameter("wsx", [BL, D], bf16, isOutput=True)

    with tile.TileContext(nc) as tc:
        with (
            tc.tile_pool(name="const", bufs=1) as cp,
            tc.tile_pool(name="e8p", bufs=3) as e8p,
            tc.tile_pool(name="e16p", bufs=3) as e16p,
            tc.tile_pool(name="encn", bufs=3) as enp,
            tc.tile_pool(name="work", bufs=2) as wp,
            tc.tile_pool(name="rows", bufs=2) as rp,
            tc.tile_pool(name="wh_ps", bufs=4, space="PSUM") as wh_ps,
            tc.tile_pool(name="ctx_ps", bufs=2, space="PSUM") as ctx_ps,
        ):
            # ---- tiny consts (DVE, before any PE work) ----
            warm = cp.tile([128, 512], bf16)
            nc.vector.memset(warm, 0.125)
            ones32 = cp.tile([32, 128], bf16)
            nc.vector.memset(ones32, 1.0)
            ones128 = cp.tile([128, 1], f32)
            nc.vector.memset(ones128, 1.0)
            scratch = cp.tile([1, 8], f32)
            # ws rows (partition 0, K=32 padded): memset the zero rows now so
            # only the single row-0 DMA is on the ws critical path later.
            rows_all = cp.tile([32, BL * D], bf16)
            nc.vector.memset(rows_all, 0.0)

            # ---- startup DMAs ----
            # gpsimd queue: ws-path inputs first (chunk-paced wbt), then the
            # WH weights; sync queue in parallel: tile(0,0) enc loads + wab.
            dect_sb = cp.tile([128, DC, BL], bf16)
            nc.gpsimd.dma_start(out=dect_sb, in_=dect_h[:, :, :])
            wbt_sb = cp.tile([128, DC, D], bf16)
            nc.gpsimd.dma_start(out=wbt_sb, in_=wbt_h[:, :, :])

            def load_tile(bj, st, pfx="", eng=None):
                eng = eng or nc.gpsimd
                s0 = st * ST
                tiles = {}
                if NP8:
                    t8 = e8p.tile([128, NP8, 2, ST], f8, tag="e8", name=f"e8{pfx}")
                    for ep in range(NP8):
                        eng.dma_start(
                            out=t8[:, ep, :, :], in_=enct8_h[:, ep, :, bj, s0:s0 + ST])
                    tiles["t8"] = t8
                if EC16:
                    t16 = e16p.tile([128, EC16, ST], bf16, tag="e16", name=f"e16{pfx}")
                    eng.dma_start(out=t16, in_=enct16_h[:, :, bj, s0:s0 + ST])
                    tiles["t16"] = t16
                tn = enp.tile([128, NSB, E2], bf16, tag="en", name=f"en{pfx}")
                eng.dma_start(out=tn, in_=encn_h[:, bj, st * NSB:(st + 1) * NSB, :])
                tiles["tn"] = tn
                return tiles

            if NP8:
                wct8_sb = cp.tile([128, NP8, 2, D], f8)
                nc.gpsimd.dma_start(out=wct8_sb, in_=wct8_h[:, :, :, :])
            if EC16:
                wct16_sb = cp.tile([128, EC16, D], bf16)
                nc.gpsimd.dma_start(out=wct16_sb, in_=wct16_h[:, :, :])
            # tile(0,0) + wab go on the otherwise-idle sync queue so their
            # descriptors generate in parallel with the weight DMAs above.
            wab_sb = cp.tile([128, D], bf16)
            nc.sync.dma_start(out=wab_sb, in_=wab_h[:, :])
            tile_cache = {(0, 0): load_tile(0, 0, "00", eng=nc.sync)}

            # ---- PE warmup: ramp the p-state during startup DMA wait ----
            if not _NO_WARM:
                wps = wh_ps.tile([128, 512], f32, tag="wh", name="warmps")
                for _ in range(6):
                    nc.tensor.matmul(wps, warm[:, 0:128], warm, start=True, stop=True)
                nc.scalar.copy(out=scratch[0:1, 0:4], in_=wps[0:1, 0:4])

            # ---- ws = dec @ Wb.T  [BL, D], then broadcast to [128, D]/batch ----
            ws_ps = [wh_ps.tile([128, 512], f32, tag="wh", name=f"wsps{h}")
                     for h in range(2)]
            for dk in range(DC):
                for h in range(2):
                    nc.tensor.matmul(
                        ws_ps[h][0:BL, :], dect_sb[:, dk, :],
                        wbt_sb[:, dk, h * 512:(h + 1) * 512],
                        start=(dk == 0), stop=(dk == DC - 1),
                    )
            ws_sb = cp.tile([BL, D], bf16)
            for h in range(2):
                nc.scalar.copy(out=ws_sb[:, h * 512:(h + 1) * 512], in_=ws_ps[h][0:BL, :])
            # matmul moving operands must start at partition 0: bounce each
            # batch's ws row through DRAM to partition 0 (padded to K=32 with
            # zeros), then replicate across partitions with a ones matmul.
            nc.gpsimd.dma_start(out=wsx_h[:, :], in_=ws_sb)
            if _NO_RT:
                nc.vector.tensor_copy(out=rows_all[0:1, 0:D], in_=ws_sb[0:1, :])
            else:
                nc.gpsimd.dma_start(
                    out=rows_all[0:1, :],
                    in_=wsx_h[:, :].rearrange("b d -> (b d)")[None, :])
            rows = [rows_all[:, bj * D:(bj + 1) * D] for bj in range(BL)]
            # per-batch ws broadcast [128, D]: emitted lazily at each batch's
            # first use so only batch 0's matmuls sit on the startup path.
            wsb = [None] * BL

            def emit_bcast(bj):
                w = cp.tile([128, D], f32, tag="wsb", bufs=BL, name=f"wsb{bj}")
                for h in range(2):
                    bc = wh_ps.tile([128, 512], f32, tag="wh", name=f"bc{bj}{h}")
                    nc.tensor.matmul(
                        bc, ones32, rows[bj][:, h * 512:(h + 1) * 512],
                        start=True, stop=True,
                    )
                    nc.vector.tensor_copy(out=w[:, h * 512:(h + 1) * 512], in_=bc)
                wsb[bj] = w

            # ---- main loop ----
            state = {}
            pending = []

            def emit_ctx(bj, st, tn, exp_all, ctx_acc):
                for eh in range(2):
                    ct = ctx_ps.tile([1, 512], f32, tag="ctx", name="ct")
                    for j in range(NSB):
                        nc.tensor.matmul(
                            ct, exp_all[:, st * NSB + j:st * NSB + j + 1],
                            tn[:, j, eh * 512:(eh + 1) * 512],
                            start=(j == 0), stop=(j == NSB - 1),
                        )
                    sl = ctx_acc[0:1, eh * 512:(eh + 1) * 512]
                    nc.vector.tensor_add(out=sl, in0=sl, in1=ct)

            def finish_batch(bj):
                st_ = state[bj]
                zcol = rp.tile([128, 1], f32, tag="zcol")
                nc.vector.reduce_sum(out=zcol, in_=st_["exp_all"], axis=X)
                zps = ctx_ps.tile([1, 512], f32, tag="ctx", name="zps")
                nc.tensor.matmul(zps[0:1, 0:1], zcol, ones128, start=True, stop=True)
                rz = rp.tile([1, 1], f32, tag="rz")
                nc.vector.reciprocal(out=rz, in_=zps[0:1, 0:1])
                ctx_out = rp.tile([1, E2], f32, tag="cout")
                nc.vector.tensor_scalar_mul(out=ctx_out, in0=st_["ctx_acc"], scalar1=rz)
                nc.sync.dma_start(out=outp[bj:bj + 1, :], in_=ctx_out)

            tidx = 0
            for bj in range(BL):
                if not _WSMM and wsb[bj] is None:
                    emit_bcast(bj)
                exp_all = rp.tile([128, NST * NSB], bf16, tag="exp")
                ctx_acc = rp.tile([1, E2], f32, tag="ctxa")
                nc.vector.memset(ctx_acc, 0.0)
                state[bj] = dict(exp_all=exp_all, ctx_acc=ctx_acc)

                for st in range(NST):
                    if tidx >= _LIMIT:
                        continue
                    tidx += 1
                    tiles = tile_cache.pop((bj, st), None) or load_tile(bj, st)
                    t8, t16, tn = tiles.get("t8"), tiles.get("t16"), tiles["tn"]
                    last_tile = (bj == BL - 1 and st == NST - 1)
                    cts = None

                    if last_tile and pending:
                        emit_ctx(*pending.pop())

                    for sb in range(NSB):
                        tw = wp.tile([128, D], bf16, tag="tw", bufs=3)
                        sc1 = rp.tile([128, 1], f32, tag="sc1", bufs=3)
                        for dh in range(2):
                            wh = wh_ps.tile([128, 512], f32, tag="wh", name="wh")
                            # snake the chunk order: consecutive groups then
                            # share the PE row-config (K=256 DR next to DR,
                            # K=128 next to K=128) -> one reconfig per group.
                            mms = (
                                [("dr", ep) for ep in range(NP8)]
                                + [("b16", c) for c in range(EC16)]
                            )
                            if dh == 1:
                                mms = mms[::-1]
                            nk = len(mms) + (1 if _WSMM else 0)
                            for k, (kind, j) in enumerate(mms):
                                if kind == "dr":
                                    nc.tensor.matmul(
                                        wh, t8[:, j, :, sb * 128:(sb + 1) * 128],
                                        wct8_sb[:, j, :, dh * 512:(dh + 1) * 512],
                                        start=(k == 0), stop=(k == nk - 1),
                                        perf_mode=DR,
                                    )
                                else:
                                    nc.tensor.matmul(
                                        wh, t16[:, j, sb * 128:(sb + 1) * 128],
                                        wct16_sb[:, j, dh * 512:(dh + 1) * 512],
                                        start=(k == 0), stop=(k == nk - 1),
                                    )
                            if _WSMM:
                                nc.tensor.matmul(
                                    wh, ones32, rows[bj][:, dh * 512:(dh + 1) * 512],
                                    start=False, stop=True,
                                )
                                src = wh
                            else:
                                x = wp.tile([128, 512], f32, tag="x", bufs=4)
                                eng = nc.gpsimd if _GPADD else nc.vector
                                eng.tensor_add(out=x, in0=wh, in1=wsb[bj][:, dh * 512:(dh + 1) * 512])
                                src = x
                            th = tw[:, dh * 512:(dh + 1) * 512]
                            nc.scalar.activation(out=th, in_=src, func=TANH)
                        # score: one fused multiply+reduce over both d-halves
                        if _AMR:
                            nc.vector.affine_mul_reduce(
                                out=tw, accum_out=sc1, in0=tw, in1=wab_sb,
                                scale=1.0, bias=0.0,
                            )
                        else:
                            nc.vector.tensor_mul(out=tw, in0=tw, in1=wab_sb)
                            nc.vector.reduce_sum(out=sc1, in_=tw, axis=X)
                        nc.scalar.activation(
                            out=exp_all[:, st * NSB + sb:st * NSB + sb + 1],
                            in_=sc1, func=EXP,
                        )
                        if sb == 0 and not last_tile and pending:
                            emit_ctx(*pending.pop())
                        if last_tile:
                            # inline the final tile's context matmuls per
                            # s-block so the tail is one block's latency, not
                            # a whole tile's.
                            if cts is None:
                                cts = [ctx_ps.tile([1, 512], f32, tag="ctx",
                                                   name=f"ctl{eh}")
                                       for eh in range(2)]
                            for eh in range(2):
                                nc.tensor.matmul(
                                    cts[eh],
                                    exp_all[:, st * NSB + sb:st * NSB + sb + 1],
                                    tn[:, sb, eh * 512:(eh + 1) * 512],
                                    start=(sb == 0), stop=(sb == NSB - 1),
                                )

                    if last_tile:
                        for eh in range(2):
                            sl = ctx_acc[0:1, eh * 512:(eh + 1) * 512]
                            nc.vector.tensor_add(out=sl, in0=sl, in1=cts[eh])
                    else:
                        pending.append((bj, st, tn, exp_all, ctx_acc))

                    if st == NST - 1 and bj > 0 and bj * NST <= _LIMIT:
                        finish_batch(bj - 1)

            if pending:
                emit_ctx(*pending.pop())
            if BL * NST <= _LIMIT:
                finish_batch(BL - 1)

    nc.finalize()
    return nc


def _prep_inputs(dec_prev_hidden, enc_outputs, Wb, Wc, Wa):
    bf = ml_dtypes.bfloat16
    f8 = ml_dtypes.float8_e4m3
    dec = np.asarray(dec_prev_hidden, dtype=np.float32)
    enc = np.asarray(enc_outputs, dtype=np.float32)
    Wb = np.asarray(Wb, dtype=np.float32)
    Wc = np.asarray(Wc, dtype=np.float32)
    Wa = np.asarray(Wa, dtype=np.float32)

    # weights, replicated
    wct = Wc.T                                           # [e, d]
    if NP8:
        wct8 = np.ascontiguousarray(
            wct[:NF8].reshape(NP8, 2, 128, D).transpose(2, 0, 1, 3)).astype(f8)
    if EC16:
        wct16 = np.ascontiguousarray(
            wct[NF8:].reshape(EC16, 128, D).transpose(1, 0, 2)).astype(bf)
    wbt = np.ascontiguousarray(
        Wb.T.reshape(DC, 128, D).transpose(1, 0, 2)).astype(bf)   # [p, c, d2]
    dect = np.ascontiguousarray(
        dec.T.reshape(DC, 128, B).transpose(1, 0, 2)).astype(bf)  # [p, c, b]
    wab = np.ascontiguousarray(np.broadcast_to(Wa[None, :], (128, D))).astype(bf)

    # enc layouts
    enct = enc.transpose(2, 1, 0)                        # [e, b, s]
    if NP8:
        enct8 = np.ascontiguousarray(
            enct[:NF8].reshape(NP8, 2, 128, B, S).transpose(2, 0, 1, 3, 4)).astype(f8)
    if EC16:
        enct16 = np.ascontiguousarray(
            enct[NF8:].reshape(EC16, 128, B, S).transpose(1, 0, 2, 3)).astype(bf)
    # natural: [p, b, jblock, e] with s = jblock*128 + p
    encn = np.ascontiguousarray(
        enc.reshape(S // 128, 128, B, E2).transpose(1, 2, 0, 3)).astype(bf)

    in_maps = []
    for i in range(NCORES):
        bsl = slice(i * BL, (i + 1) * BL)
        m = {
            "encn": np.ascontiguousarray(encn[:, bsl]),
            "dect": np.ascontiguousarray(dect[:, :, bsl]),
            "wbt": wbt,
            "wab": wab,
        }
        if NP8:
            m["enct8"] = np.ascontiguousarray(enct8[:, :, :, bsl])
            m["wct8"] = wct8
        if EC16:
            m["enct16"] = np.ascontiguousarray(enct16[:, :, bsl])
            m["wct16"] = wct16
        in_maps.append(m)
    return in_maps


def _run(inputs, trace=False):
    from concourse.bass_utils import run_bass_kernel_spmd

    if "nc" not in _CACHE:
        _CACHE["nc"] = _build_nc()
    nc = _CACHE["nc"]
    in_maps = _prep_inputs(**inputs)
    res = run_bass_kernel_spmd(nc, in_maps, list(range(NCORES)), trace=trace)
    out = np.concatenate([res.results[i]["out"] for i in range(NCORES)], axis=0)
    return out[None, :, :].astype(np.float32), res


def kernel(dec_prev_hidden, enc_outputs, Wb, Wc, Wa):
    out, _ = _run(dict(
        dec_prev_hidden=dec_prev_hidden, enc_outputs=enc_outputs,
        Wb=Wb, Wc=Wc, Wa=Wa,
    ))
    return out


# revision 32
# speedup vs baseline: 1.1370x; 1.1370x over previous
"""Bahdanau attention on 8 Trainium2 NeuronCores (Bass/Tile).

Problem:  S=2048, B=32, D=1024, E2=1024
  ws  = dec @ Wb.T                       [B, D]
  WH  = enc @ Wc.T                       [S, B, D]
  sc  = tanh(WH + ws) . Wa               [S, B]
  at  = softmax(sc, axis=0)              [S, B]
  out = einsum('sb,sbe->be', at, enc)[None]   [1, B, 2E]

Sharding: data-parallel over batch B across 8 cores (4 batches/core);
weights replicated. Softmax axis (S) stays core-local.

Dataflow (v2, transposed WH):
  WH is computed TRANSPOSED per s-tile: psum [128s, 512d] with the enc^T
  chunk as the stationary operand and Wc^T as the moving operand. With s on
  partitions and d on the free axis, the whole score reduction moves off
  the PE: ws is added as a free-axis row broadcast (DVE), tanh on ACT, and
  score[s] = sum_d Wa_d tanh(...) is a fused multiply+reduce on DVE
  (tensor_tensor_reduce with chained accumulate across the two d-halves).
  exp then lands directly in column form [128s, 1] (ACT, with accum_out
  providing softmax-Z partials), which feeds the context matmuls as lhsT
  with no transpose/columnize step. The context contraction accumulates
  unnormalized in psum over the four 128-s blocks, folded into an SBUF
  accumulator by DVE, scaled once by 1/Z per batch.

Precision: everything bf16 except psum/accumulators (f32). The first NF8
e-dims of the WH contraction run as fp8e4 (e4m3) pairs with
MatmulPerfMode.DoubleRow, which processes K=256 per matmul at the same
per-matmul cost as K=128 bf16 (HW-measured 259 ns/MM for both at N=512,
LDWEIGHTS fully hidden). exp needs no max-subtraction: |score| <= ~26 so
exp fits fp32/bf16 range comfortably and softmax is shift-invariant.

The PE is the bottleneck (>90% busy); instruction emission order doubles
as the schedule: each tile's context matmuls are emitted after the NEXT
tile's WH matmuls so the PE never waits on the ACT exp; ~20 warmup
matmuls on garbage data ramp the PE p-state during the startup DMAs.
"""

import numpy as np
import ml_dtypes

S, B, D, E2 = 2048, 32, 1024, 1024
NCORES = 8
BL = B // NCORES          # batches per core
ST = 512                  # s-tile size
NST = S // ST             # s-tiles per batch
NSB = ST // 128           # 128-row s-blocks per s-tile
DC = D // 128             # d chunks (ws path)

import os
NF8 = int(os.environ.get("K_NF8", "512"))  # leading e-dims in fp8 DoubleRow
NP8 = NF8 // 256          # fp8 k-pairs
EC16 = (E2 - NF8) // 128  # bf16 e-chunks
_NO_TTR = bool(int(os.environ.get("K_NO_TTR", "1")))  # ttr crashes HW runtime
_NO_WARM = bool(int(os.environ.get("K_NO_WARM", "0")))
_NO_RT = bool(int(os.environ.get("K_NO_RT", "0")))    # skip DRAM ws round-trip
_LIMIT = int(os.environ.get("K_LIMIT", "99"))         # max tiles emitted
_AMR = bool(int(os.environ.get("K_AMR", "1")))        # fused affine_mul_reduce
_GPADD = bool(int(os.environ.get("K_GPADD", "0")))    # ws-add on gpsimd
_WSMM = bool(int(os.environ.get("K_WSMM", "0")))      # fold ws into psum via PE

_CACHE = {}


def _build_nc():
    import concourse.bacc as bacc
    import concourse.tile as tile
    from concourse import mybir

    f32 = mybir.dt.float32
    bf16 = mybir.dt.bfloat16
    f8 = mybir.dt.float8e4
    TANH = mybir.ActivationFunctionType.Tanh
    EXP = mybir.ActivationFunctionType.Exp
    X = mybir.AxisListType.X
    MUL = mybir.AluOpType.mult
    ADD = mybir.AluOpType.add
    DR = mybir.MatmulPerfMode.DoubleRow

    nc = bacc.Bacc()
    # host-prepped layouts (see _prep_inputs)
    if NP8:
        enct8_h = nc.declare_dram_parameter("enct8", [128, NP8, 2, BL, S], f8, isOutput=False)
        wct8_h = nc.declare_dram_parameter("wct8", [128, NP8, 2, D], f8, isOutput=False)
    if EC16:
        enct16_h = nc.declare_dram_parameter("enct16", [128, EC16, BL, S], bf16, isOutput=False)
        wct16_h = nc.declare_dram_parameter("wct16", [128, EC16, D], bf16, isOutput=False)
    encn_h = nc.declare_dram_parameter("encn", [128, BL, S // 128, E2], bf16, isOutput=False)
    dect_h = nc.declare_dram_parameter("dect", [128, DC, BL], bf16, isOutput=False)
    wbt_h = nc.declare_dram_parameter("wbt", [128, DC, D], bf16, isOutput=False)
    wab_h = nc.declare_dram_parameter("wab", [128, D], bf16, isOutput=False)
    outp = nc.declare_dram_parameter("out", [BL, E2], f32, isOutput=True)
    wsx_h = nc.declare_dram_parameter("wsx", [BL, D], bf16, isOutput=True)

    with tile.TileContext(nc) as tc:
        with (
            tc.tile_pool(name="const", bufs=1) as cp,
            tc.tile_pool(name="e8p", bufs=3) as e8p,
            tc.tile_pool(name="e16p", bufs=3) as e16p,
            tc.tile_pool(name="encn", bufs=3) as enp,
            tc.tile_pool(name="work", bufs=2) as wp,
            tc.tile_pool(name="rows", bufs=2) as rp,
            tc.tile_pool(name="wh_ps", bufs=4, space="PSUM") as wh_ps,
            tc.tile_pool(name="ctx_ps", bufs=2, space="PSUM") as ctx_ps,
        ):
            # ---- tiny consts (DVE, before any PE work) ----
            warm = cp.tile([128, 512], bf16)
            nc.vector.memset(warm, 0.125)
            ones32 = cp.tile([32, 128], bf16)
            nc.vector.memset(ones32, 1.0)
            ones128 = cp.tile([128, 1], f32)
            nc.vector.memset(ones128, 1.0)
            scratch = cp.tile([1, 8], f32)
            # ws rows (partition 0, K=32 padded): memset the zero rows now so
            # only the single row-0 DMA is on the ws critical path later.
            rows_all = cp.tile([32, BL * D], bf16)
            nc.vector.memset(rows_all, 0.0)

            # ---- startup DMAs ----
            # gpsimd queue: ws-path inputs first (chunk-paced wbt), then the
            # WH weights; sync queue in parallel: tile(0,0) enc loads + wab.
            dect_sb = cp.tile([128, DC, BL], bf16)
            nc.gpsimd.dma_start(out=dect_sb, in_=dect_h[:, :, :])
            wbt_sb = cp.tile([128, DC, D], bf16)
            nc.gpsimd.dma_start(out=wbt_sb, in_=wbt_h[:, :, :])

            def load_tile(bj, st, pfx="", eng=None):
                eng = eng or nc.gpsimd
                s0 = st * ST
                tiles = {}
                if NP8:
                    t8 = e8p.tile([128, NP8, 2, ST], f8, tag="e8", name=f"e8{pfx}")
                    for ep in range(NP8):
                        eng.dma_start(
                            out=t8[:, ep, :, :], in_=enct8_h[:, ep, :, bj, s0:s0 + ST])
                    tiles["t8"] = t8
                if EC16:
                    t16 = e16p.tile([128, EC16, ST], bf16, tag="e16", name=f"e16{pfx}")
                    eng.dma_start(out=t16, in_=enct16_h[:, :, bj, s0:s0 + ST])
                    tiles["t16"] = t16
                tn = enp.tile([128, NSB, E2], bf16, tag="en", name=f"en{pfx}")
                eng.dma_start(out=tn, in_=encn_h[:, bj, st * NSB:(st + 1) * NSB, :])
                tiles["tn"] = tn
                return tiles

            if NP8:
                wct8_sb = cp.tile([128, NP8, 2, D], f8)
                nc.gpsimd.dma_start(out=wct8_sb, in_=wct8_h[:, :, :, :])
            if EC16:
                wct16_sb = cp.tile([128, EC16, D], bf16)
                nc.gpsimd.dma_start(out=wct16_sb, in_=wct16_h[:, :, :])
            # tile(0,0) + wab go on the otherwise-idle sync queue so their
            # descriptors generate in parallel with the weight DMAs above.
            wab_sb = cp.tile([128, D], bf16)
            nc.sync.dma_start(out=wab_sb, in_=wab_h[:, :])
            tile_cache = {(0, 0): load_tile(0, 0, "00", eng=nc.sync)}

            # ---- PE warmup: ramp the p-state during startup DMA wait ----
            if not _NO_WARM:
                wps = wh_ps.tile([128, 512], f32, tag="wh", name="warmps")
                for _ in range(6):
                    nc.tensor.matmul(wps, warm[:, 0:128], warm, start=True, stop=True)
                nc.scalar.copy(out=scratch[0:1, 0:4], in_=wps[0:1, 0:4])

            # ---- ws = dec @ Wb.T  [BL, D], then broadcast to [128, D]/batch ----
            ws_ps = [wh_ps.tile([128, 512], f32, tag="wh", name=f"wsps{h}")
                     for h in range(2)]
            for dk in range(DC):
                for h in range(2):
                    nc.tensor.matmul(
                        ws_ps[h][0:BL, :], dect_sb[:, dk, :],
                        wbt_sb[:, dk, h * 512:(h + 1) * 512],
                        start=(dk == 0), stop=(dk == DC - 1),
                    )
            ws_sb = cp.tile([BL, D], bf16)
            for h in range(2):
                nc.scalar.copy(out=ws_sb[:, h * 512:(h + 1) * 512], in_=ws_ps[h][0:BL, :])
            # matmul moving operands must start at partition 0: bounce each
            # batch's ws row through DRAM to partition 0 (padded to K=32 with
            # zeros), then replicate across partitions with a ones matmul.
            nc.gpsimd.dma_start(out=wsx_h[:, :], in_=ws_sb)
            if _NO_RT:
                nc.vector.tensor_copy(out=rows_all[0:1, 0:D], in_=ws_sb[0:1, :])
            else:
                nc.gpsimd.dma_start(
                    out=rows_all[0:1, :],
                    in_=wsx_h[:, :].rearrange("b d -> (b d)")[None, :])
            rows = [rows_all[:, bj * D:(bj + 1) * D] for bj in range(BL)]
            # per-batch ws broadcast [128, D]: emitted lazily at each batch's
            # first use so only batch 0's matmuls sit on the startup path.
            wsb = [None] * BL

            def emit_bcast(bj):
                w = cp.tile([128, D], f32, tag="wsb", bufs=BL, name=f"wsb{bj}")
                for h in range(2):
                    bc = wh_ps.tile([128, 512], f32, tag="wh", name=f"bc{bj}{h}")
                    nc.tensor.matmul(
                        bc, ones32, rows[bj][:, h * 512:(h + 1) * 512],
                        start=True, stop=True,
                    )
                    nc.vector.tensor_copy(out=w[:, h * 512:(h + 1) * 512], in_=bc)
                wsb[bj] = w

            # ---- main loop ----
            state = {}
            pending = []

            def emit_ctx(bj, st, tn, exp_all, ctx_acc):
                for eh in range(2):
                    ct = ctx_ps.tile([1, 512], f32, tag="ctx", name="ct")
                    for j in range(NSB):
                        nc.tensor.matmul(
                            ct, exp_all[:, st * NSB + j:st * NSB + j + 1],
                            tn[:, j, eh * 512:(eh + 1) * 512],
                            start=(j == 0), stop=(j == NSB - 1),
                        )
                    sl = ctx_acc[0:1, eh * 512:(eh + 1) * 512]
                    nc.vector.tensor_add(out=sl, in0=sl, in1=ct)

            def finish_batch(bj):
                st_ = state[bj]
                zcol = rp.tile([128, 1], f32, tag="zcol")
                nc.vector.reduce_sum(out=zcol, in_=st_["exp_all"], axis=X)
                zps = ctx_ps.tile([1, 512], f32, tag="ctx", name="zps")
                nc.tensor.matmul(zps[0:1, 0:1], zcol, ones128, start=True, stop=True)
                rz = rp.tile([1, 1], f32, tag="rz")
                nc.vector.reciprocal(out=rz, in_=zps[0:1, 0:1])
                ctx_out = rp.tile([1, E2], f32, tag="cout")
                nc.vector.tensor_scalar_mul(out=ctx_out, in0=st_["ctx_acc"], scalar1=rz)
                nc.sync.dma_start(out=outp[bj:bj + 1, :], in_=ctx_out)

            tidx = 0
            for bj in range(BL):
                if not _WSMM and wsb[bj] is None:
                    emit_bcast(bj)
                exp_all = rp.tile([128, NST * NSB], bf16, tag="exp")
                ctx_acc = rp.tile([1, E2], f32, tag="ctxa")
                nc.vector.memset(ctx_acc, 0.0)
                state[bj] = dict(exp_all=exp_all, ctx_acc=ctx_acc)

                for st in range(NST):
                    if tidx >= _LIMIT:
                        continue
                    tidx += 1
                    tiles = tile_cache.pop((bj, st), None) or load_tile(bj, st)
                    t8, t16, tn = tiles.get("t8"), tiles.get("t16"), tiles["tn"]
                    last_tile = (bj == BL - 1 and st == NST - 1)
                    cts = None

                    if last_tile and pending:
                        emit_ctx(*pending.pop())

                    for sb in range(NSB):
                        tw = wp.tile([128, D], bf16, tag="tw", bufs=3)
                        sc1 = rp.tile([128, 1], f32, tag="sc1", bufs=3)
                        for dh in range(2):
                            wh = wh_ps.tile([128, 512], f32, tag="wh", name="wh")
                            # snake the chunk order: consecutive groups then
                            # share the PE row-config (K=256 DR next to DR,
                            # K=128 next to K=128) -> one reconfig per group.
                            mms = (
                                [("dr", ep) for ep in range(NP8)]
                                + [("b16", c) for c in range(EC16)]
                            )
                            if dh == 1:
                                mms = mms[::-1]
                            nk = len(mms) + (1 if _WSMM else 0)
                            for k, (kind, j) in enumerate(mms):
                                if kind == "dr":
                                    nc.tensor.matmul(
                                        wh, t8[:, j, :, sb * 128:(sb + 1) * 128],
                                        wct8_sb[:, j, :, dh * 512:(dh + 1) * 512],
                                        start=(k == 0), stop=(k == nk - 1),
                                        perf_mode=DR,
                                    )
                                else:
                                    nc.tensor.matmul(
                                        wh, t16[:, j, sb * 128:(sb + 1) * 128],
                                        wct16_sb[:, j, dh * 512:(dh + 1) * 512],
                                        start=(k == 0), stop=(k == nk - 1),
                                    )
                            if _WSMM:
                                nc.tensor.matmul(
                                    wh, ones32, rows[bj][:, dh * 512:(dh + 1) * 512],
                                    start=False, stop=True,
                                )
                                src = wh
                            else:
                                x = wp.tile([128, 512], f32, tag="x", bufs=4)
                                eng = nc.gpsimd if _GPADD else nc.vector
                                eng.tensor_add(out=x, in0=wh, in1=wsb[bj][:, dh * 512:(dh + 1) * 512])
                                src = x
                            th = tw[:, dh * 512:(dh + 1) * 512]
                            nc.scalar.activation(out=th, in_=src, func=TANH)
                        # score: one fused multiply+reduce over both d-halves
                        if _AMR:
                            nc.vector.affine_mul_reduce(
                                out=tw, accum_out=sc1, in0=tw, in1=wab_sb,
                                scale=1.0, bias=0.0,
                            )
                        else:
                            nc.vector.tensor_mul(out=tw, in0=tw, in1=wab_sb)
                            nc.vector.reduce_sum(out=sc1, in_=tw, axis=X)
                        nc.scalar.activation(
                            out=exp_all[:, st * NSB + sb:st * NSB + sb + 1],
                            in_=sc1, func=EXP,
                        )
                        if sb == 0 and not last_tile and pending:
                            emit_ctx(*pending.pop())
                        if last_tile:
                            # inline the final tile's context matmuls per
                            # s-block so the tail is one block's latency, not
                            # a whole tile's.
                            if cts is None:
                                cts = [ctx_ps.tile([1, 512], f32, tag="ctx",
                                                   name=f"ctl{eh}")
                                       for eh in range(2)]
                            for eh in range(2):
                                nc.tensor.matmul(
                                    cts[eh],
                                    exp_all[:, st * NSB + sb:st * NSB + sb + 1],
                                    tn[:, sb, eh * 512:(eh + 1) * 512],
                                    start=(sb == 0), stop=(sb == NSB - 1),
                                )

                    if last_tile:
                        for eh in range(2):
                            sl = ctx_acc[0:1, eh * 512:(eh + 1) * 512]
                            nc.vector.tensor_add(out=sl, in0=sl, in1=cts[eh])
                    else:
                        pending.append((bj, st, tn, exp_all, ctx_acc))

                    if st == NST - 1 and bj > 0 and bj * NST <= _LIMIT:
                        finish_batch(bj - 1)

            if pending:
                emit_ctx(*pending.pop())
            if BL * NST <= _LIMIT:
                finish_batch(BL - 1)

    nc.finalize()
    return nc


def _prep_inputs(dec_prev_hidden, enc_outputs, Wb, Wc, Wa):
    bf = ml_dtypes.bfloat16
    f8 = ml_dtypes.float8_e4m3
    dec = np.asarray(dec_prev_hidden, dtype=np.float32)
    enc = np.asarray(enc_outputs, dtype=np.float32)
    Wb = np.asarray(Wb, dtype=np.float32)
    Wc = np.asarray(Wc, dtype=np.float32)
    Wa = np.asarray(Wa, dtype=np.float32)

    # weights, replicated
    wct = Wc.T                                           # [e, d]
    if NP8:
        wct8 = np.ascontiguousarray(
            wct[:NF8].reshape(NP8, 2, 128, D).transpose(2, 0, 1, 3)).astype(f8)
    if EC16:
        wct16 = np.ascontiguousarray(
            wct[NF8:].reshape(EC16, 128, D).transpose(1, 0, 2)).astype(bf)
    wbt = np.ascontiguousarray(
        Wb.T.reshape(DC, 128, D).transpose(1, 0, 2)).astype(bf)   # [p, c, d2]
    dect = np.ascontiguousarray(
        dec.T.reshape(DC, 128, B).transpose(1, 0, 2)).astype(bf)  # [p, c, b]
    wab = np.ascontiguousarray(np.broadcast_to(Wa[None, :], (128, D))).astype(bf)

    # enc layouts
    enct = enc.transpose(2, 1, 0)                        # [e, b, s]
    if NP8:
        enct8 = np.ascontiguousarray(
            enct[:NF8].reshape(NP8, 2, 128, B, S).transpose(2, 0, 1, 3, 4)).astype(f8)
    if EC16:
        enct16 = np.ascontiguousarray(
            enct[NF8:].reshape(EC16, 128, B, S).transpose(1, 0, 2, 3)).astype(bf)
    # natural: [p, b, jblock, e] with s = jblock*128 + p
    encn = np.ascontiguousarray(
        enc.reshape(S // 128, 128, B, E2).transpose(1, 2, 0, 3)).astype(bf)

    in_maps = []
    for i in range(NCORES):
        bsl = slice(i * BL, (i + 1) * BL)
        m = {
            "encn": np.ascontiguousarray(encn[:, bsl]),
            "dect": np.ascontiguousarray(dect[:, :, bsl]),
            "wbt": wbt,
            "wab": wab,
        }
        if NP8:
            m["enct8"] = np.ascontiguousarray(enct8[:, :, :, bsl])
            m["wct8"] = wct8
        if EC16:
            m["enct16"] = np.ascontiguousarray(enct16[:, :, bsl])
            m["wct16"] = wct16
        in_maps.append(m)
    return in_maps


def _run(inputs, trace=False):
    from concourse.bass_utils import run_bass_kernel_spmd

    if "nc" not in _CACHE:
        _CACHE["nc"] = _build_nc()
    nc = _CACHE["nc"]
    in_maps = _prep_inputs(**inputs)
    res = run_bass_kernel_spmd(nc, in_maps, list(range(NCORES)), trace=trace)
    out = np.concatenate([res.results[i]["out"] for i in range(NCORES)], axis=0)
    return out[None, :, :].astype(np.float32), res


def kernel(dec_prev_hidden, enc_outputs, Wb, Wc, Wa):
    out, _ = _run(dict(
        dec_prev_hidden=dec_prev_hidden, enc_outputs=enc_outputs,
        Wb=Wb, Wc=Wc, Wa=Wa,
    ))
    return out


# revision 35
# speedup vs baseline: 1.1436x; 1.0058x over previous
"""Bahdanau attention on 8 Trainium2 NeuronCores (Bass/Tile).

Problem:  S=2048, B=32, D=1024, E2=1024
  ws  = dec @ Wb.T                       [B, D]
  WH  = enc @ Wc.T                       [S, B, D]
  sc  = tanh(WH + ws) . Wa               [S, B]
  at  = softmax(sc, axis=0)              [S, B]
  out = einsum('sb,sbe->be', at, enc)[None]   [1, B, 2E]

Sharding: data-parallel over batch B across 8 cores (4 batches/core);
weights replicated. Softmax axis (S) stays core-local.

Dataflow (v2, transposed WH):
  WH is computed TRANSPOSED per s-tile: psum [128s, 512d] with the enc^T
  chunk as the stationary operand and Wc^T as the moving operand. With s on
  partitions and d on the free axis, the whole score reduction moves off
  the PE: ws is added as a free-axis row broadcast (DVE), tanh on ACT, and
  score[s] = sum_d Wa_d tanh(...) is a fused multiply+reduce on DVE
  (tensor_tensor_reduce with chained accumulate across the two d-halves).
  exp then lands directly in column form [128s, 1] (ACT, with accum_out
  providing softmax-Z partials), which feeds the context matmuls as lhsT
  with no transpose/columnize step. The context contraction accumulates
  unnormalized in psum over the four 128-s blocks, folded into an SBUF
  accumulator by DVE, scaled once by 1/Z per batch.

Precision: everything bf16 except psum/accumulators (f32). The first NF8
e-dims of the WH contraction run as fp8e4 (e4m3) pairs with
MatmulPerfMode.DoubleRow, which processes K=256 per matmul at the same
per-matmul cost as K=128 bf16 (HW-measured 259 ns/MM for both at N=512,
LDWEIGHTS fully hidden). exp needs no max-subtraction: |score| <= ~26 so
exp fits fp32/bf16 range comfortably and softmax is shift-invariant.

The PE is the bottleneck (>90% busy); instruction emission order doubles
as the schedule: each tile's context matmuls are emitted after the NEXT
tile's WH matmuls so the PE never waits on the ACT exp; ~20 warmup
matmuls on garbage data ramp the PE p-state during the startup DMAs.
"""

import numpy as np
import ml_dtypes

S, B, D, E2 = 2048, 32, 1024, 1024
NCORES = 8
BL = B // NCORES          # batches per core
ST = 512                  # s-tile size
NST = S // ST             # s-tiles per batch
NSB = ST // 128           # 128-row s-blocks per s-tile
DC = D // 128             # d chunks (ws path)

import os
NF8 = int(os.environ.get("K_NF8", "512"))  # leading e-dims in fp8 DoubleRow
NP8 = NF8 // 256          # fp8 k-pairs
EC16 = (E2 - NF8) // 128  # bf16 e-chunks
_NO_TTR = bool(int(os.environ.get("K_NO_TTR", "1")))  # ttr crashes HW runtime
_NO_WARM = bool(int(os.environ.get("K_NO_WARM", "0")))
_NO_RT = bool(int(os.environ.get("K_NO_RT", "0")))    # skip DRAM ws round-trip
_LIMIT = int(os.environ.get("K_LIMIT", "99"))         # max tiles emitted
_AMR = bool(int(os.environ.get("K_AMR", "1")))        # fused affine_mul_reduce
_GPADD = bool(int(os.environ.get("K_GPADD", "0")))    # ws-add on gpsimd
_WSMM = bool(int(os.environ.get("K_WSMM", "0")))      # fold ws into psum via PE

_CACHE = {}


def _build_nc():
    import concourse.bacc as bacc
    import concourse.tile as tile
    from concourse import mybir

    f32 = mybir.dt.float32
    bf16 = mybir.dt.bfloat16
    f8 = mybir.dt.float8e4
    TANH = mybir.ActivationFunctionType.Tanh
    EXP = mybir.ActivationFunctionType.Exp
    X = mybir.AxisListType.X
    MUL = mybir.AluOpType.mult
    ADD = mybir.AluOpType.add
    DR = mybir.MatmulPerfMode.DoubleRow

    nc = bacc.Bacc()
    # host-prepped layouts (see _prep_inputs)
    if NP8:
        enct8_h = nc.declare_dram_parameter("enct8", [128, NP8, 2, BL, S], f8, isOutput=False)
        wct8_h = nc.declare_dram_parameter("wct8", [128, NP8, 2, D], f8, isOutput=False)
    if EC16:
        enct16_h = nc.declare_dram_parameter("enct16", [128, EC16, BL, S], bf16, isOutput=False)
        wct16_h = nc.declare_dram_parameter("wct16", [128, EC16, D], bf16, isOutput=False)
    encn_h = nc.declare_dram_parameter("encn", [128, BL, S // 128, E2], bf16, isOutput=False)
    dect_h = nc.declare_dram_parameter("dect", [128, DC, BL], bf16, isOutput=False)
    wbt_h = nc.declare_dram_parameter("wbt", [128, DC, D], bf16, isOutput=False)
    wab_h = nc.declare_dram_parameter("wab", [128, D], bf16, isOutput=False)
    outp = nc.declare_dram_parameter("out", [BL, E2], f32, isOutput=True)
    wsx_h = nc.declare_dram_parameter("wsx", [BL, D], bf16, isOutput=True)

    with tile.TileContext(nc) as tc:
        with (
            tc.tile_pool(name="const", bufs=1) as cp,
            tc.tile_pool(name="e8p", bufs=3) as e8p,
            tc.tile_pool(name="e16p", bufs=3) as e16p,
            tc.tile_pool(name="encn", bufs=3) as enp,
            tc.tile_pool(name="work", bufs=2) as wp,
            tc.tile_pool(name="rows", bufs=2) as rp,
            tc.tile_pool(name="wh_ps", bufs=4, space="PSUM") as wh_ps,
            tc.tile_pool(name="ctx_ps", bufs=2, space="PSUM") as ctx_ps,
        ):
            # ---- tiny consts (DVE, before any PE work) ----
            warm = cp.tile([128, 512], bf16)
            nc.vector.memset(warm, 0.125)
            ones32 = cp.tile([32, 128], bf16)
            nc.vector.memset(ones32, 1.0)
            ones128 = cp.tile([128, 1], f32)
            nc.vector.memset(ones128, 1.0)
            scratch = cp.tile([1, 8], f32)
            # ws rows (partition 0, K=32 padded): memset the zero rows now so
            # only the single row-0 DMA is on the ws critical path later.
            rows_all = cp.tile([32, BL * D], bf16)
            nc.vector.memset(rows_all, 0.0)

            # ---- startup DMAs ----
            # Three DMA queues (~180 GB/s each): scalar+sync (HWDGE) carry
            # the ws path (wbt halves) so the ws matmuls start earliest; the
            # gpsimd SWDGE ring carries the Wc weights + first encn tile.
            dect_sb = cp.tile([128, DC, BL], bf16)
            nc.scalar.dma_start(out=dect_sb, in_=dect_h[:, :, :])
            wbt_sb = cp.tile([128, DC, D], bf16)
            nc.scalar.dma_start(out=wbt_sb[:, 0:DC // 2, :], in_=wbt_h[:, 0:DC // 2, :])
            nc.sync.dma_start(out=wbt_sb[:, DC // 2:, :], in_=wbt_h[:, DC // 2:, :])

            def load_tile(bj, st, pfx=""):
                s0 = st * ST
                tiles = {}
                if NP8:
                    t8 = e8p.tile([128, NP8, 2, ST], f8, tag="e8", name=f"e8{pfx}")
                    for ep in range(NP8):
                        nc.sync.dma_start(
                            out=t8[:, ep, :, :], in_=enct8_h[:, ep, :, bj, s0:s0 + ST])
                    tiles["t8"] = t8
                if EC16:
                    t16 = e16p.tile([128, EC16, ST], bf16, tag="e16", name=f"e16{pfx}")
                    nc.sync.dma_start(out=t16, in_=enct16_h[:, :, bj, s0:s0 + ST])
                    tiles["t16"] = t16
                tn = enp.tile([128, NSB, E2], bf16, tag="en", name=f"en{pfx}")
                nc.gpsimd.dma_start(out=tn, in_=encn_h[:, bj, st * NSB:(st + 1) * NSB, :])
                tiles["tn"] = tn
                return tiles

            if NP8:
                wct8_sb = cp.tile([128, NP8, 2, D], f8)
                nc.gpsimd.dma_start(out=wct8_sb, in_=wct8_h[:, :, :, :])
            if EC16:
                wct16_sb = cp.tile([128, EC16, D], bf16)
                nc.gpsimd.dma_start(out=wct16_sb, in_=wct16_h[:, :, :])
            wab_sb = cp.tile([128, D], bf16)
            nc.sync.dma_start(out=wab_sb, in_=wab_h[:, :])
            tile_cache = {(0, 0): load_tile(0, 0, "00")}

            # ---- PE warmup: ramp the p-state during startup DMA wait ----
            if not _NO_WARM:
                wps = wh_ps.tile([128, 512], f32, tag="wh", name="warmps")
                for _ in range(6):
                    nc.tensor.matmul(wps, warm[:, 0:128], warm, start=True, stop=True)
                nc.scalar.copy(out=scratch[0:1, 0:4], in_=wps[0:1, 0:4])

            # ---- ws = dec @ Wb.T  [BL, D], then broadcast to [128, D]/batch ----
            ws_ps = [wh_ps.tile([128, 512], f32, tag="wh", name=f"wsps{h}")
                     for h in range(2)]
            for dk in range(DC):
                for h in range(2):
                    nc.tensor.matmul(
                        ws_ps[h][0:BL, :], dect_sb[:, dk, :],
                        wbt_sb[:, dk, h * 512:(h + 1) * 512],
                        start=(dk == 0), stop=(dk == DC - 1),
                    )
            ws_sb = cp.tile([BL, D], bf16)
            for h in range(2):
                nc.scalar.copy(out=ws_sb[:, h * 512:(h + 1) * 512], in_=ws_ps[h][0:BL, :])
            # matmul moving operands must start at partition 0: bounce each
            # batch's ws row through DRAM to partition 0 (padded to K=32 with
            # zeros), then replicate across partitions with a ones matmul.
            # round-trip on the scalar HWDGE queue: same engine as the ws
            # copies above, so no cross-queue barrier sits in front of it.
            nc.scalar.dma_start(out=wsx_h[:, :], in_=ws_sb)
            if _NO_RT:
                nc.vector.tensor_copy(out=rows_all[0:1, 0:D], in_=ws_sb[0:1, :])
            else:
                nc.scalar.dma_start(
                    out=rows_all[0:1, :],
                    in_=wsx_h[:, :].rearrange("b d -> (b d)")[None, :])
            rows = [rows_all[:, bj * D:(bj + 1) * D] for bj in range(BL)]
            # per-batch ws broadcast [128, D]: emitted lazily at each batch's
            # first use so only batch 0's matmuls sit on the startup path.
            wsb = [None] * BL

            def emit_bcast(bj):
                w = cp.tile([128, D], f32, tag="wsb", bufs=BL, name=f"wsb{bj}")
                for h in range(2):
                    bc = wh_ps.tile([128, 512], f32, tag="wh", name=f"bc{bj}{h}")
                    nc.tensor.matmul(
                        bc, ones32, rows[bj][:, h * 512:(h + 1) * 512],
                        start=True, stop=True,
                    )
                    nc.vector.tensor_copy(out=w[:, h * 512:(h + 1) * 512], in_=bc)
                wsb[bj] = w

            # ---- main loop ----
            state = {}
            pending = []

            def emit_ctx(bj, st, tn, exp_all, ctx_acc):
                for eh in range(2):
                    ct = ctx_ps.tile([1, 512], f32, tag="ctx", name="ct")
                    for j in range(NSB):
                        nc.tensor.matmul(
                            ct, exp_all[:, st * NSB + j:st * NSB + j + 1],
                            tn[:, j, eh * 512:(eh + 1) * 512],
                            start=(j == 0), stop=(j == NSB - 1),
                        )
                    sl = ctx_acc[0:1, eh * 512:(eh + 1) * 512]
                    nc.vector.tensor_add(out=sl, in0=sl, in1=ct)

            def finish_batch(bj):
                st_ = state[bj]
                zcol = rp.tile([128, 1], f32, tag="zcol")
                nc.vector.reduce_sum(out=zcol, in_=st_["exp_all"], axis=X)
                zps = ctx_ps.tile([1, 512], f32, tag="ctx", name="zps")
                nc.tensor.matmul(zps[0:1, 0:1], zcol, ones128, start=True, stop=True)
                rz = rp.tile([1, 1], f32, tag="rz")
                nc.vector.reciprocal(out=rz, in_=zps[0:1, 0:1])
                ctx_out = rp.tile([1, E2], f32, tag="cout")
                nc.vector.tensor_scalar_mul(out=ctx_out, in0=st_["ctx_acc"], scalar1=rz)
                nc.scalar.dma_start(out=outp[bj:bj + 1, :], in_=ctx_out)

            tidx = 0
            for bj in range(BL):
                if not _WSMM and wsb[bj] is None:
                    emit_bcast(bj)
                exp_all = rp.tile([128, NST * NSB], bf16, tag="exp")
                ctx_acc = rp.tile([1, E2], f32, tag="ctxa")
                nc.vector.memset(ctx_acc, 0.0)
                state[bj] = dict(exp_all=exp_all, ctx_acc=ctx_acc)

                for st in range(NST):
                    if tidx >= _LIMIT:
                        continue
                    tidx += 1
                    tiles = tile_cache.pop((bj, st), None) or load_tile(bj, st)
                    t8, t16, tn = tiles.get("t8"), tiles.get("t16"), tiles["tn"]
                    last_tile = (bj == BL - 1 and st == NST - 1)
                    cts = None

                    if last_tile and pending:
                        emit_ctx(*pending.pop())

                    for sb in range(NSB):
                        tw = wp.tile([128, D], bf16, tag="tw", bufs=3)
                        sc1 = rp.tile([128, 1], f32, tag="sc1", bufs=3)
                        for dh in range(2):
                            wh = wh_ps.tile([128, 512], f32, tag="wh", name="wh")
                            # snake the chunk order: consecutive groups then
                            # share the PE row-config (K=256 DR next to DR,
                            # K=128 next to K=128) -> one reconfig per group.
                            mms = (
                                [("dr", ep) for ep in range(NP8)]
                                + [("b16", c) for c in range(EC16)]
                            )
                            if dh == 1:
                                mms = mms[::-1]
                            nk = len(mms) + (1 if _WSMM else 0)
                            for k, (kind, j) in enumerate(mms):
                                if kind == "dr":
                                    nc.tensor.matmul(
                                        wh, t8[:, j, :, sb * 128:(sb + 1) * 128],
                                        wct8_sb[:, j, :, dh * 512:(dh + 1) * 512],
                                        start=(k == 0), stop=(k == nk - 1),
                                        perf_mode=DR,
                                    )
                                else:
                                    nc.tensor.matmul(
                                        wh, t16[:, j, sb * 128:(sb + 1) * 128],
                                        wct16_sb[:, j, dh * 512:(dh + 1) * 512],
                                        start=(k == 0), stop=(k == nk - 1),
                                    )
                            if _WSMM:
                                nc.tensor.matmul(
                                    wh, ones32, rows[bj][:, dh * 512:(dh + 1) * 512],
                                    start=False, stop=True,
                                )
                                src = wh
                            else:
                                x = wp.tile([128, 512], f32, tag="x", bufs=4)
                                eng = nc.gpsimd if _GPADD else nc.vector
                                eng.tensor_add(out=x, in0=wh, in1=wsb[bj][:, dh * 512:(dh + 1) * 512])
                                src = x
                            th = tw[:, dh * 512:(dh + 1) * 512]
                            nc.scalar.activation(out=th, in_=src, func=TANH)
                        # score: one fused multiply+reduce over both d-halves
                        if _AMR:
                            nc.vector.affine_mul_reduce(
                                out=tw, accum_out=sc1, in0=tw, in1=wab_sb,
                                scale=1.0, bias=0.0,
                            )
                        else:
                            nc.vector.tensor_mul(out=tw, in0=tw, in1=wab_sb)
                            nc.vector.reduce_sum(out=sc1, in_=tw, axis=X)
                        nc.scalar.activation(
                            out=exp_all[:, st * NSB + sb:st * NSB + sb + 1],
                            in_=sc1, func=EXP,
                        )
                        if sb == 0 and not last_tile and pending:
                            emit_ctx(*pending.pop())
                        if last_tile:
                            # inline the final tile's context matmuls per
                            # s-block so the tail is one block's latency, not
                            # a whole tile's.
                            if cts is None:
                                cts = [ctx_ps.tile([1, 512], f32, tag="ctx",
                                                   name=f"ctl{eh}")
                                       for eh in range(2)]
                            for eh in range(2):
                                nc.tensor.matmul(
                                    cts[eh],
                                    exp_all[:, st * NSB + sb:st * NSB + sb + 1],
                                    tn[:, sb, eh * 512:(eh + 1) * 512],
                                    start=(sb == 0), stop=(sb == NSB - 1),
                                )

                    if last_tile:
                        for eh in range(2):
                            sl = ctx_acc[0:1, eh * 512:(eh + 1) * 512]
                            nc.vector.tensor_add(out=sl, in0=sl, in1=cts[eh])
                    else:
                        pending.append((bj, st, tn, exp_all, ctx_acc))

                    if st == NST - 1 and bj > 0 and bj * NST <= _LIMIT:
                        finish_batch(bj - 1)

            if pending:
                emit_ctx(*pending.pop())
            if BL * NST <= _LIMIT:
                finish_batch(BL - 1)

    nc.finalize()
    return nc


def _prep_inputs(dec_prev_hidden, enc_outputs, Wb, Wc, Wa):
    bf = ml_dtypes.bfloat16
    f8 = ml_dtypes.float8_e4m3
    dec = np.asarray(dec_prev_hidden, dtype=np.float32)
    enc = np.asarray(enc_outputs, dtype=np.float32)
    Wb = np.asarray(Wb, dtype=np.float32)
    Wc = np.asarray(Wc, dtype=np.float32)
    Wa = np.asarray(Wa, dtype=np.float32)

    # weights, replicated
    wct = Wc.T                                           # [e, d]
    if NP8:
        wct8 = np.ascontiguousarray(
            wct[:NF8].reshape(NP8, 2, 128, D).transpose(2, 0, 1, 3)).astype(f8)
    if EC16:
        wct16 = np.ascontiguousarray(
            wct[NF8:].reshape(EC16, 128, D).transpose(1, 0, 2)).astype(bf)
    wbt = np.ascontiguousarray(
        Wb.T.reshape(DC, 128, D).transpose(1, 0, 2)).astype(bf)   # [p, c, d2]
    dect = np.ascontiguousarray(
        dec.T.reshape(DC, 128, B).transpose(1, 0, 2)).astype(bf)  # [p, c, b]
    wab = np.ascontiguousarray(np.broadcast_to(Wa[None, :], (128, D))).astype(bf)

    # enc layouts
    enct = enc.transpose(2, 1, 0)                        # [e, b, s]
    if NP8:
        enct8 = np.ascontiguousarray(
            enct[:NF8].reshape(NP8, 2, 128, B, S).transpose(2, 0, 1, 3, 4)).astype(f8)
    if EC16:
        enct16 = np.ascontiguousarray(
            enct[NF8:].reshape(EC16, 128, B, S).transpose(1, 0, 2, 3)).astype(bf)
    # natural: [p, b, jblock, e] with s = jblock*128 + p
    encn = np.ascontiguousarray(
        enc.reshape(S // 128, 128, B, E2).transpose(1, 2, 0, 3)).astype(bf)

    in_maps = []
    for i in range(NCORES):
        bsl = slice(i * BL, (i + 1) * BL)
        m = {
            "encn": np.ascontiguousarray(encn[:, bsl]),
            "dect": np.ascontiguousarray(dect[:, :, bsl]),
            "wbt": wbt,
            "wab": wab,
        }
        if NP8:
            m["enct8"] = np.ascontiguousarray(enct8[:, :, :, bsl])
            m["wct8"] = wct8
        if EC16:
            m["enct16"] = np.ascontiguousarray(enct16[:, :, bsl])
            m["wct16"] = wct16
        in_maps.append(m)
    return in_maps


def _run(inputs, trace=False):
    from concourse.bass_utils import run_bass_kernel_spmd

    if "nc" not in _CACHE:
        _CACHE["nc"] = _build_nc()
    nc = _CACHE["nc"]
    in_maps = _prep_inputs(**inputs)
    res = run_bass_kernel_spmd(nc, in_maps, list(range(NCORES)), trace=trace)
    out = np.concatenate([res.results[i]["out"] for i in range(NCORES)], axis=0)
    return out[None, :, :].astype(np.float32), res


def kernel(dec_prev_hidden, enc_outputs, Wb, Wc, Wa):
    out, _ = _run(dict(
        dec_prev_hidden=dec_prev_hidden, enc_outputs=enc_outputs,
        Wb=Wb, Wc=Wc, Wa=Wa,
    ))
    return out
